# revision 38
# baseline (speedup 1.0000x reference)
"""Multi-head attention (B=2, S=2048, E=1024, H=16, DH=64) on 8 TRN2 NeuronCores.

Sharding: core c handles batch c//4 and heads [4*(c%4), 4*(c%4)+4).
Each core computes its QKV projection slice and full attention for its 4
heads locally; host gathers the per-core [S, 256] outputs into [B, S, E].

Per-core kernel layout choices:
  - x is pre-transposed on host to xT [E, S]; W slices pre-transposed to
    [E, F].  QKV projection runs in two passes:
      pass 1 (transposed out): qkT [f, s] for Q,K — psum[f=128, s=512] =
        sum_e WqkT[e, f].T @ xT[e, s]; bias is added during the psum->sbuf
        copy as a per-partition tensor_scalar_add (bias varies along f).
      pass 2 (natural out): v [s, f] — psum[s=128, f=256] = bias seed +
        sum_e xT[e, s].T @ WvT[e, f]; bias seeded via a K=1 matmul
        (ones[1,128].T @ bv[1,256]).
  - Heads are paired in qkT tiles: tile rows 0:64 = head 2i, 64:128 = head
    2i+1.  Score matmuls slice partitions [0:64] or [64:128] of both
    operands (contraction d=64, tile_position rows 0/64).
  - Scores are computed transposed: sT[k, q] = K_tile.T @ QT, so softmax's
    denominator comes for free from the AV matmul via a ones column
    appended to V ([V | 1], N=65).  exp runs on ScalarE straight out of
    PSUM with scale=1/8 folded in; no max-subtraction (scores/8 are ~N(0,
    0.33), far from overflow).
  - AV: av[q, 0:65] += expT[k, q].T @ [V|1][k, 0:65] accumulated over 16
    k-tiles in PSUM; final normalize out[q, d] = av[q, d] * recip(av[q, 64])
    is a per-partition tensor_scalar_mul on VectorE.
"""

import os
from contextlib import ExitStack

import ml_dtypes
import numpy as np

import concourse.bass as bass
import concourse.mybir as mybir
import concourse.tile as tile
from concourse.bass import ts
from concourse.bass_utils import run_bass_kernel_spmd

B, S, E, H, DH = 2, 2048, 1024, 16, 64
NCORES = 8
HPC = H // (NCORES // B)  # heads per core = 4
FQK = HPC * 2 * DH  # 512: [Q_h0|K_h0|Q_h1|K_h1|...] pairs -> see layout below
FV = HPC * DH  # 256
ECH = E // 128  # 8 contraction chunks
ST = S // 128  # 16 s-tiles
BF16 = mybir.dt.bfloat16
F32 = mybir.dt.float32

_CACHED = {}


def _build_nc(
    loop: int | None = None,
    skip_qkv: bool = False,
    skip_exp: bool = False,
    skip_av: bool = False,
) -> bass.Bass:
    nc = bass.Bass(trn_type="TRN2")
    xT = nc.dram_tensor("xT", [E, S], BF16, kind="ExternalInput")
    wqkT = nc.dram_tensor("wqkT", [E, FQK], BF16, kind="ExternalInput")
    wvT = nc.dram_tensor("wvT", [E, FV], BF16, kind="ExternalInput")
    # Q bias, pre-divided by sqrt(DH), head-pair layout (see _host_shards).
    # K bias is dropped entirely: adding b_k shifts every score in a softmax
    # row by a k-independent constant, which softmax cancels.  The Q bias
    # enters scores only through (b_q . K)[k], a per-k scalar we feed to the
    # exp activation as its per-partition bias AP.
    bq8 = nc.dram_tensor("bq8", [128, HPC // 2], BF16, kind="ExternalInput")
    bv = nc.dram_tensor("bv", [1, FV], BF16, kind="ExternalInput")
    out = nc.dram_tensor("out", [S, FV], F32, kind="ExternalOutput")

    QKT = FQK // 128  # 4 f-tiles for the Q/K pass; tile i holds heads 2i, 2i+1
    with TileCtx(nc) as tc, ExitStack() as ctx:
        if loop is not None:
            ctx.enter_context(tc.For_i(0, loop, 1))
        consts = ctx.enter_context(tc.tile_pool(name="consts", bufs=1))
        xt_pool = ctx.enter_context(tc.tile_pool(name="xt", bufs=1))
        qk_pool = ctx.enter_context(tc.tile_pool(name="qk", bufs=1))
        v_pool = ctx.enter_context(tc.tile_pool(name="v", bufs=1))
        out_pool = ctx.enter_context(tc.tile_pool(name="outp", bufs=1))
        expt_pool = ctx.enter_context(tc.tile_pool(name="expt", bufs=26))
        misc_pool = ctx.enter_context(tc.tile_pool(name="misc", bufs=2))

        # ---- constants / weights to SBUF ----
        ones = consts.tile([1, 128], BF16, tag="ones")
        nc.vector.memset(ones[:], 1.0)
        bq8_sb = consts.tile([128, HPC // 2], BF16, tag="bq8")
        nc.sync.dma_start(bq8_sb[:], bq8[:, :])
        bv_sb = consts.tile([1, FV], BF16, tag="bv")
        nc.sync.dma_start(bv_sb[:], bv[:, :])
        wqk_sb = consts.tile([128, ECH, FQK], BF16, tag="wqk")
        nc.sync.dma_start(wqk_sb[:], wqkT.rearrange("(c p) f -> p c f", p=128))
        wv_sb = consts.tile([128, ECH, FV], BF16, tag="wv")
        nc.sync.dma_start(wv_sb[:], wvT.rearrange("(c p) f -> p c f", p=128))

        xt_sb = xt_pool.tile([128, ECH, S], BF16, tag="xt")
        xt_r = xT.rearrange("(c p) s -> p c s", p=128)
        for c in range(ECH):
            nc.sync.dma_start(xt_sb[:, c, :], xt_r[:, c, :])

        # qkT storage: tile i [128, S]: rows 0:64 head 2i's Q^T (or K^T),
        # rows 64:128 head 2i+1's.  qk_sb[i] i in 0..3: [Q pair0, Q pair1,
        # K pair0, K pair1]  -- matches wqkT column layout (see host prep).
        qk_sb = qk_pool.tile([128, QKT, S], BF16, tag="qkt")
        if skip_qkv:
            nc.vector.memset(qk_sb[:, :, 0:2], 0.5)
        # v_sb: [128(k within tile), st, head, 65]; col 64 = 1.0
        v_sb = v_pool.tile([128, ST, HPC, DH + 1], BF16, tag="vsb")
        nc.vector.memset(v_sb[:, :, :, DH], 1.0)
        out_sb = out_pool.tile([128, ST, FV], F32, tag="outsb")
        if skip_av:
            nc.vector.memset(out_sb[:, 0, 0:2], 0.0)

        with tc.tile_pool(name="mmps", bufs=3, space="PSUM") as mmps:
            # pass 1: Q/K transposed.  Pair-0 tiles (ft 0, 2) first so head
            # 0's scores can start while pair 1 is still projecting.
            for ft in [0, 2, 1, 3] if not skip_qkv else []:
                for sc in range(S // 512):
                    ps = mmps.tile([128, 512], F32, tag="mm")
                    for e in range(ECH):
                        nc.tensor.matmul(
                            ps[:],
                            lhsT=wqk_sb[:, e, ts(ft, 128)],
                            rhs=xt_sb[:, e, ts(sc, 512)],
                            start=(e == 0),
                            stop=(e == ECH - 1),
                        )
                    nc.vector.tensor_copy(qk_sb[:, ft, ts(sc, 512)], ps[:])

        def emit_vpass(vmm, st):
            # pass 2 (V natural), one s-tile: seeded with the V bias.
            ps2 = vmm.tile([128, FV], F32, tag="vmm")
            nc.tensor.matmul(ps2[:], lhsT=ones[:], rhs=bv_sb[:], start=True, stop=False)
            for e in range(ECH):
                nc.tensor.matmul(
                    ps2[:],
                    lhsT=xt_sb[:, e, ts(st, 128)],
                    rhs=wv_sb[:, e, :],
                    start=False,
                    stop=(e == ECH - 1),
                )
            nc.vector.tensor_copy(
                v_sb[:, st, :, 0:DH], ps2.rearrange("p (h d) -> p h d", d=DH)
            )

        def head_slices(h):
            pair, lo = divmod(h, 2)
            qsl = slice(64 * lo, 64 * lo + 64)
            return pair, qsl

        def emit_bqk(scps, h):
            # (b_q . K)[k] / sqrt(DH): one [128,1] column per k-tile.
            pair, qsl = head_slices(h)
            k_t = qk_sb[qsl, 2 + pair, :]
            bqk_ps = scps.tile([128, ST], F32, tag="sc")
            for kt in range(ST):
                nc.tensor.matmul(
                    bqk_ps[:, kt : kt + 1],
                    lhsT=k_t[:, ts(kt, 128)],
                    rhs=bq8_sb[qsl, pair : pair + 1],
                    start=True,
                    stop=True,
                )
            bqk8 = misc_pool.tile([128, ST], F32, tag="bqk8", bufs=2)
            nc.vector.tensor_copy(bqk8[:], bqk_ps[:])
            return bqk8

        def emit_scores_exp(scps, h, kt, bqk8, expt):
            pair, qsl = head_slices(h)
            q_t = qk_sb[qsl, pair, :]
            k_t = qk_sb[qsl, 2 + pair, :]
            for qh in range(2):
                sc_ps = scps.tile([128, 1024], F32, tag="sc")
                for qq in range(2):
                    nc.tensor.matmul(
                        sc_ps[:, ts(qq, 512)],
                        lhsT=k_t[:, ts(kt, 128)],
                        rhs=q_t[:, ts(2 * qh + qq, 512)],
                        start=True,
                        stop=True,
                    )
                if not skip_exp:
                    nc.scalar.activation(
                        expt[:, ts(qh, 1024)],
                        sc_ps[:],
                        mybir.ActivationFunctionType.Exp,
                        bias=bqk8[:, kt : kt + 1],
                        scale=float(1.0 / np.sqrt(DH)),
                    )

        def emit_av(av, h, expts, qts):
            # qt-outer / kt-inner: PSUM has_written clears at bank
            # granularity on start=True, so accumulation groups sharing a
            # bank must run one-at-a-time, not interleaved.
            for qt in qts:
                for kt in range(ST):
                    nc.tensor.matmul(
                        av[:, qt, 0 : DH + 1],
                        lhsT=expts[kt][:, ts(qt, 128)],
                        rhs=v_sb[:, kt, h, :],
                        start=(kt == 0),
                        stop=(kt == ST - 1),
                    )

        def emit_normalize(av, h):
            rec = misc_pool.tile([128, ST], F32, tag="rec", bufs=2)
            nc.vector.reciprocal(rec[:], av[:, :, DH])
            # Copy then in-place multiply: keeps each DVE instruction's
            # sync waits on a single proc (PE for the copy, DVE for the
            # mul) — DVE ops only have one HW wait slot.
            for qt in range(ST):
                nc.vector.tensor_copy(out_sb[:, qt, ts(h, DH)], av[:, qt, 0:DH])
                nc.vector.tensor_mul(
                    out_sb[:, qt, ts(h, DH)],
                    out_sb[:, qt, ts(h, DH)],
                    rec[:, qt : qt + 1].broadcast_to((128, DH)),
                )

        # Attention, software-pipelined one head deep: head h's ACT-bound
        # exp loop hides head h-1's AV matmuls (2 qt-groups per kt over the
        # first 8 kt) and, for h=0, the V projection pass (one s-tile per
        # kt).  PSUM: sc 2x2 banks + (vmm 2 banks during h0 | av 4 banks
        # from h1 on) <= 8.
        with tc.tile_pool(name="scps", bufs=2, space="PSUM") as scps:
            inner = ExitStack()
            vmm = inner.enter_context(tc.tile_pool(name="vmm", bufs=2, space="PSUM"))
            avps = None
            expts_by_head = {}
            av_by_head = {}
            for h in range(HPC):
                bqk8 = emit_bqk(scps, h)
                expts_by_head[h] = []
                prev = h - 1 if h > 0 and not skip_av else None
                if prev is not None:
                    av_by_head[prev] = avps.tile([128, ST, 128], F32, tag="av", name="av")
                for kt in range(ST):
                    expt = expt_pool.tile([128, S], BF16, tag="expt")
                    expts_by_head[h].append(expt)
                    if skip_exp:
                        nc.vector.memset(expt[:, 0:2], 0.5)
                    emit_scores_exp(scps, h, kt, bqk8, expt)
                    if h == 0 and not skip_qkv:
                        emit_vpass(vmm, kt)
                    if prev is not None and kt < 8:
                        emit_av(
                            av_by_head[prev], prev, expts_by_head[prev], [2 * kt, 2 * kt + 1]
                        )
                    if prev is not None and kt == 8:
                        emit_normalize(av_by_head[prev], prev)
                        del expts_by_head[prev]
                if h == 0:
                    # vmm's 2 banks are released before avps' 4 are claimed.
                    inner.close()
                    inner = ExitStack()
                    avps = inner.enter_context(
                        tc.tile_pool(name="avps", bufs=1, space="PSUM")
                    )
            if not skip_av:
                last = HPC - 1
                av_by_head[last] = avps.tile([128, ST, 128], F32, tag="av", name="av")
                emit_av(av_by_head[last], last, expts_by_head[last], list(range(ST)))
                emit_normalize(av_by_head[last], last)
            inner.close()

        nc.sync.dma_start(out.rearrange("(t p) f -> p t f", p=128), out_sb[:])
    _split_multiwaits(nc)
    return nc


_NO_SPLIT = ("InstEventSemaphore", "InstCall", "InstUnconditionalBranch")


def _split_multiwaits(nc: bass.Bass):
    """Walrus codegen rejects TPB instructions carrying >1 sync wait ("Too
    many sync wait commands"), but Tile's wait assigner can emit 2-3 when an
    instruction depends on several procs.  Hoist all but the last wait onto
    engine-level InstEventSemaphore carriers (the same construct Tile's own
    barriers use) inserted just before the instruction."""
    seq = 0
    for blk in nc.m.functions[0].blocks:
        out_insts = []
        for inst in blk.instructions:
            si = inst.sync_info
            waits = list(si.on_wait) if si is not None and si.on_wait else []
            if len(waits) > 1 and type(inst).__name__ not in _NO_SPLIT:
                for w in waits[:-1]:
                    carrier = mybir.InstEventSemaphore(
                        name=f"bass_waitsplit_{seq}",
                        engine=inst.engine,
                        ins=[],
                        outs=[],
                        sync_info=mybir.SyncInfo(on_wait=[w], on_update=[]),
                    )
                    seq += 1
                    out_insts.append(carrier)
                inst.sync_info = mybir.SyncInfo(
                    on_wait=[waits[-1]], on_update=list(si.on_update or [])
                )
            out_insts.append(inst)
        blk.instructions = out_insts


def TileCtx(nc):
    return tile.TileContext(nc)


def _host_shards(x, W_qkv, b_qkv):
    """Build the 8 per-core input maps (numpy, host-side)."""
    x = np.asarray(x, dtype=np.float32)
    W = np.asarray(W_qkv, dtype=np.float32)
    bias = np.asarray(b_qkv, dtype=np.float32)
    xTs = [np.ascontiguousarray(x[b].T).astype(ml_dtypes.bfloat16) for b in range(B)]
    in_maps = []
    for c in range(NCORES):
        b, g = divmod(c, NCORES // B)  # batch, head-group
        h0 = HPC * g
        # wqkT columns: [Q_h0|Q_h1 .. interleaved by PAIR]: tile i (128 cols)
        # = [Q_{h0+2i}(64) | Q_{h0+2i+1}(64)] for i=0,1 then K pairs.
        qcols, kcols, bqcols = [], [], []
        for i in range(HPC // 2):
            for j in range(2):
                h = h0 + 2 * i + j
                qcols.append(W[DH * h : DH * (h + 1)])
                kcols.append(W[E + DH * h : E + DH * (h + 1)])
        for i in range(HPC // 2):
            bqcols.append(
                np.concatenate(
                    [
                        bias[DH * (h0 + 2 * i) : DH * (h0 + 2 * i + 1)],
                        bias[DH * (h0 + 2 * i + 1) : DH * (h0 + 2 * i + 2)],
                    ]
                )
            )
        wqk = np.concatenate(qcols + kcols, axis=0)  # [512, E]
        wqkT = np.ascontiguousarray(wqk.T).astype(ml_dtypes.bfloat16)
        wv = W[2 * E + DH * h0 : 2 * E + DH * (h0 + HPC)]  # [256, E]
        wvT = np.ascontiguousarray(wv.T).astype(ml_dtypes.bfloat16)
        bq8 = (np.stack(bqcols, axis=1) / np.sqrt(DH)).astype(ml_dtypes.bfloat16)
        bv = bias[2 * E + DH * h0 : 2 * E + DH * (h0 + HPC)].reshape(1, FV)
        in_maps.append(
            {
                "xT": xTs[b],
                "wqkT": wqkT,
                "wvT": wvT,
                "bq8": bq8,
                "bv": bv.astype(ml_dtypes.bfloat16),
            }
        )
    return in_maps


LAST_EXEC_NS = None


def kernel(x, W_qkv, b_qkv):
    global LAST_EXEC_NS
    in_maps = _host_shards(x, W_qkv, b_qkv)
    if "nc" not in _CACHED:
        _CACHED["nc"] = _build_nc()
    trace = bool(int(os.environ.get("BASS_KERNEL_TRACE", "0")))
    res = run_bass_kernel_spmd(
        _CACHED["nc"], in_maps, core_ids=list(range(NCORES)), trace=trace
    )
    LAST_EXEC_NS = res.exec_time_ns
    out = np.zeros((B, S, E), dtype=np.float32)
    for c in range(NCORES):
        b, g = divmod(c, NCORES // B)
        out[b, :, FV * g : FV * (g + 1)] = res.results[c]["out"]
    return out


# revision 40
# speedup vs baseline: 1.2023x; 1.2023x over previous
"""Multi-head attention (B=2, S=2048, E=1024, H=16, DH=64) on 8 TRN2 NeuronCores.

Sharding: core c handles batch c//4 and heads [4*(c%4), 4*(c%4)+4).
Each core computes its QKV projection slice and full attention for its 4
heads locally; host gathers the per-core [S, 256] outputs into [B, S, E].

Per-core kernel layout choices:
  - x is pre-transposed on host to xT [E, S]; W slices pre-transposed to
    [E, F].  QKV projection runs in two passes:
      pass 1 (transposed out): qkT [f, s] for Q,K — psum[f=128, s=512] =
        sum_e WqkT[e, f].T @ xT[e, s]; bias is added during the psum->sbuf
        copy as a per-partition tensor_scalar_add (bias varies along f).
      pass 2 (natural out): v [s, f] — psum[s=128, f=256] = bias seed +
        sum_e xT[e, s].T @ WvT[e, f]; bias seeded via a K=1 matmul
        (ones[1,128].T @ bv[1,256]).
  - Heads are paired in qkT tiles: tile rows 0:64 = head 2i, 64:128 = head
    2i+1.  Score matmuls slice partitions [0:64] or [64:128] of both
    operands (contraction d=64, tile_position rows 0/64).
  - Scores are computed transposed: sT[k, q] = K_tile.T @ QT, so softmax's
    denominator comes for free from the AV matmul via a ones column
    appended to V ([V | 1], N=65).  exp runs on ScalarE straight out of
    PSUM with scale=1/8 folded in; no max-subtraction (scores/8 are ~N(0,
    0.33), far from overflow).
  - AV: av[q, 0:65] += expT[k, q].T @ [V|1][k, 0:65] accumulated over 16
    k-tiles in PSUM; final normalize out[q, d] = av[q, d] * recip(av[q, 64])
    is a per-partition tensor_scalar_mul on VectorE.
"""

import os
from contextlib import ExitStack

import ml_dtypes
import numpy as np

import concourse.bass as bass
import concourse.mybir as mybir
import concourse.tile as tile
from concourse.bass import ts
from concourse.bass_utils import run_bass_kernel_spmd

B, S, E, H, DH = 2, 2048, 1024, 16, 64
NCORES = 8
HPC = H // (NCORES // B)  # heads per core = 4
FQK = HPC * 2 * DH  # 512: [Q_h0|K_h0|Q_h1|K_h1|...] pairs -> see layout below
FV = HPC * DH  # 256
ECH = E // 128  # 8 contraction chunks
ST = S // 128  # 16 s-tiles
BF16 = mybir.dt.bfloat16
F32 = mybir.dt.float32

_CACHED = {}


def _build_nc(
    loop: int | None = None,
    skip_qkv: bool = False,
    skip_exp: bool = False,
    skip_av: bool = False,
    interleave_av: bool = True,
) -> bass.Bass:
    nc = bass.Bass(trn_type="TRN2")
    xT = nc.dram_tensor("xT", [E, S], BF16, kind="ExternalInput")
    wqkT = nc.dram_tensor("wqkT", [E, FQK], BF16, kind="ExternalInput")
    wvT = nc.dram_tensor("wvT", [E, FV], BF16, kind="ExternalInput")
    # Q bias, pre-divided by sqrt(DH), head-pair layout (see _host_shards).
    # K bias is dropped entirely: adding b_k shifts every score in a softmax
    # row by a k-independent constant, which softmax cancels.  The Q bias
    # enters scores only through (b_q . K)[k], a per-k scalar we feed to the
    # exp activation as its per-partition bias AP.
    bq8 = nc.dram_tensor("bq8", [128, HPC // 2], BF16, kind="ExternalInput")
    bv = nc.dram_tensor("bv", [1, FV], BF16, kind="ExternalInput")
    out = nc.dram_tensor("out", [S, FV], F32, kind="ExternalOutput")

    QKT = FQK // 128  # 4 f-tiles for the Q/K pass; tile i holds heads 2i, 2i+1
    with TileCtx(nc) as tc, ExitStack() as ctx:
        if loop is not None:
            ctx.enter_context(tc.For_i(0, loop, 1))
        consts = ctx.enter_context(tc.tile_pool(name="consts", bufs=1))
        xt_pool = ctx.enter_context(tc.tile_pool(name="xt", bufs=1))
        qk_pool = ctx.enter_context(tc.tile_pool(name="qk", bufs=1))
        v_pool = ctx.enter_context(tc.tile_pool(name="v", bufs=1))
        out_pool = ctx.enter_context(tc.tile_pool(name="outp", bufs=1))
        expt_pool = ctx.enter_context(tc.tile_pool(name="expt", bufs=26))
        misc_pool = ctx.enter_context(tc.tile_pool(name="misc", bufs=2))

        # ---- constants / weights to SBUF ----
        ones = consts.tile([1, 128], BF16, tag="ones")
        nc.vector.memset(ones[:], 1.0)
        bq8_sb = consts.tile([128, HPC // 2], BF16, tag="bq8")
        nc.sync.dma_start(bq8_sb[:], bq8[:, :])
        bv_sb = consts.tile([1, FV], BF16, tag="bv")
        nc.sync.dma_start(bv_sb[:], bv[:, :])
        wqk_sb = consts.tile([128, ECH, FQK], BF16, tag="wqk")
        nc.sync.dma_start(wqk_sb[:], wqkT.rearrange("(c p) f -> p c f", p=128))
        wv_sb = consts.tile([128, ECH, FV], BF16, tag="wv")
        nc.sync.dma_start(wv_sb[:], wvT.rearrange("(c p) f -> p c f", p=128))

        xt_sb = xt_pool.tile([128, ECH, S], BF16, tag="xt")
        xt_r = xT.rearrange("(c p) s -> p c s", p=128)
        for c in range(ECH):
            nc.sync.dma_start(xt_sb[:, c, :], xt_r[:, c, :])

        # qkT storage: tile i [128, S]: rows 0:64 head 2i's Q^T (or K^T),
        # rows 64:128 head 2i+1's.  qk_sb[i] i in 0..3: [Q pair0, Q pair1,
        # K pair0, K pair1]  -- matches wqkT column layout (see host prep).
        qk_sb = qk_pool.tile([128, QKT, S], BF16, tag="qkt")
        if skip_qkv:
            nc.vector.memset(qk_sb[:, :, 0:2], 0.5)
        # v_sb: [128(k within tile), st, head, 65]; col 64 = 1.0
        v_sb = v_pool.tile([128, ST, HPC, DH + 1], BF16, tag="vsb")
        nc.vector.memset(v_sb[:, :, :, DH], 1.0)
        out_sb = out_pool.tile([128, ST, FV], F32, tag="outsb")
        if skip_av:
            nc.vector.memset(out_sb[:, 0, 0:2], 0.0)

        with tc.tile_pool(name="mmps", bufs=3, space="PSUM") as mmps:
            # pass 1: Q/K transposed.  Pair-0 tiles (ft 0, 2) first so head
            # 0's scores can start while pair 1 is still projecting.
            for ft in [0, 2, 1, 3] if not skip_qkv else []:
                for sc in range(S // 512):
                    ps = mmps.tile([128, 512], F32, tag="mm")
                    for e in range(ECH):
                        nc.tensor.matmul(
                            ps[:],
                            lhsT=wqk_sb[:, e, ts(ft, 128)],
                            rhs=xt_sb[:, e, ts(sc, 512)],
                            start=(e == 0),
                            stop=(e == ECH - 1),
                        )
                    nc.vector.tensor_copy(qk_sb[:, ft, ts(sc, 512)], ps[:])

        def emit_vpass(vmm, st):
            # pass 2 (V natural), one s-tile: seeded with the V bias.
            ps2 = vmm.tile([128, FV], F32, tag="vmm")
            nc.tensor.matmul(ps2[:], lhsT=ones[:], rhs=bv_sb[:], start=True, stop=False)
            for e in range(ECH):
                nc.tensor.matmul(
                    ps2[:],
                    lhsT=xt_sb[:, e, ts(st, 128)],
                    rhs=wv_sb[:, e, :],
                    start=False,
                    stop=(e == ECH - 1),
                )
            nc.vector.tensor_copy(
                v_sb[:, st, :, 0:DH], ps2.rearrange("p (h d) -> p h d", d=DH)
            )

        def head_slices(h):
            pair, lo = divmod(h, 2)
            qsl = slice(64 * lo, 64 * lo + 64)
            return pair, qsl

        def emit_bqk(scps, h):
            # (b_q . K)[k] / sqrt(DH): one [128,1] column per k-tile.
            pair, qsl = head_slices(h)
            k_t = qk_sb[qsl, 2 + pair, :]
            bqk_ps = scps.tile([128, ST], F32, tag="sc")
            for kt in range(ST):
                nc.tensor.matmul(
                    bqk_ps[:, kt : kt + 1],
                    lhsT=k_t[:, ts(kt, 128)],
                    rhs=bq8_sb[qsl, pair : pair + 1],
                    start=True,
                    stop=True,
                )
            bqk8 = misc_pool.tile([128, ST], F32, tag="bqk8", bufs=2)
            nc.vector.tensor_copy(bqk8[:], bqk_ps[:])
            return bqk8

        def emit_scores_exp(scps, h, kt, bqk8, expt):
            pair, qsl = head_slices(h)
            q_t = qk_sb[qsl, pair, :]
            k_t = qk_sb[qsl, 2 + pair, :]
            for qh in range(2):
                sc_ps = scps.tile([128, 1024], F32, tag="sc")
                for qq in range(2):
                    nc.tensor.matmul(
                        sc_ps[:, ts(qq, 512)],
                        lhsT=k_t[:, ts(kt, 128)],
                        rhs=q_t[:, ts(2 * qh + qq, 512)],
                        start=True,
                        stop=True,
                    )
                if not skip_exp:
                    nc.scalar.activation(
                        expt[:, ts(qh, 1024)],
                        sc_ps[:],
                        mybir.ActivationFunctionType.Exp,
                        bias=bqk8[:, kt : kt + 1],
                        scale=float(1.0 / np.sqrt(DH)),
                    )

        def emit_av(av, h, expts, qts):
            # qt-outer / kt-inner: PSUM has_written clears at bank
            # granularity on start=True, so accumulation groups sharing a
            # bank must run one-at-a-time, not interleaved.
            for qt in qts:
                for kt in range(ST):
                    nc.tensor.matmul(
                        av[:, qt, 0 : DH + 1],
                        lhsT=expts[kt][:, ts(qt, 128)],
                        rhs=v_sb[:, kt, h, :],
                        start=(kt == 0),
                        stop=(kt == ST - 1),
                    )

        def emit_normalize(av, h):
            rec = misc_pool.tile([128, ST], F32, tag="rec", bufs=2)
            nc.vector.reciprocal(rec[:], av[:, :, DH])
            # Copy then in-place multiply: keeps each DVE instruction's
            # sync waits on a single proc (PE for the copy, DVE for the
            # mul) — DVE ops only have one HW wait slot.
            for qt in range(ST):
                nc.vector.tensor_copy(out_sb[:, qt, ts(h, DH)], av[:, qt, 0:DH])
                nc.vector.tensor_mul(
                    out_sb[:, qt, ts(h, DH)],
                    out_sb[:, qt, ts(h, DH)],
                    rec[:, qt : qt + 1].broadcast_to((128, DH)),
                )

        # Attention, software-pipelined one head deep: head h's ACT-bound
        # exp loop hides head h-1's AV matmuls (2 qt-groups per kt over the
        # first 8 kt) and, for h=0, the V projection pass (one s-tile per
        # kt).  PSUM: sc 2x2 banks + (vmm 2 banks during h0 | av 4 banks
        # from h1 on) <= 8.
        with tc.tile_pool(name="scps", bufs=2, space="PSUM") as scps:
            inner = ExitStack()
            vmm = inner.enter_context(tc.tile_pool(name="vmm", bufs=2, space="PSUM"))
            avps = None
            expts_by_head = {}
            av_by_head = {}
            for h in range(HPC):
                bqk8 = emit_bqk(scps, h)
                expts_by_head[h] = []
                prev = h - 1 if h > 0 and interleave_av and not skip_av else None
                if prev is not None:
                    av_by_head[prev] = avps.tile([128, ST, 128], F32, tag="av", name="av")
                for kt in range(ST):
                    expt = expt_pool.tile([128, S], BF16, tag="expt")
                    expts_by_head[h].append(expt)
                    if skip_exp:
                        nc.vector.memset(expt[:, 0:2], 0.5)
                    emit_scores_exp(scps, h, kt, bqk8, expt)
                    if h == 0 and not skip_qkv:
                        emit_vpass(vmm, kt)
                    if prev is not None and kt < 8:
                        emit_av(
                            av_by_head[prev], prev, expts_by_head[prev], [2 * kt, 2 * kt + 1]
                        )
                    if prev is not None and kt == 8:
                        emit_normalize(av_by_head[prev], prev)
                        del expts_by_head[prev]
                if h == 0:
                    # vmm's 2 banks are released before avps' 4 are claimed.
                    inner.close()
                    inner = ExitStack()
                    avps = inner.enter_context(
                        tc.tile_pool(name="avps", bufs=1, space="PSUM")
                    )
                if not interleave_av and not skip_av:
                    av_by_head[h] = avps.tile([128, ST, 128], F32, tag="av", name="av")
                    emit_av(av_by_head[h], h, expts_by_head[h], list(range(ST)))
                    emit_normalize(av_by_head[h], h)
                    del expts_by_head[h]
            if interleave_av and not skip_av:
                last = HPC - 1
                av_by_head[last] = avps.tile([128, ST, 128], F32, tag="av", name="av")
                emit_av(av_by_head[last], last, expts_by_head[last], list(range(ST)))
                emit_normalize(av_by_head[last], last)
            inner.close()

        nc.sync.dma_start(out.rearrange("(t p) f -> p t f", p=128), out_sb[:])
    _split_multiwaits(nc)
    return nc


_NO_SPLIT = ("InstEventSemaphore", "InstCall", "InstUnconditionalBranch")


def _split_multiwaits(nc: bass.Bass):
    """Walrus codegen rejects TPB instructions carrying >1 sync wait ("Too
    many sync wait commands"), but Tile's wait assigner can emit 2-3 when an
    instruction depends on several procs.  Hoist all but the last wait onto
    engine-level InstEventSemaphore carriers (the same construct Tile's own
    barriers use) inserted just before the instruction."""
    seq = 0
    for blk in nc.m.functions[0].blocks:
        out_insts = []
        for inst in blk.instructions:
            si = inst.sync_info
            waits = list(si.on_wait) if si is not None and si.on_wait else []
            if len(waits) > 1 and type(inst).__name__ not in _NO_SPLIT:
                for w in waits[:-1]:
                    carrier = mybir.InstEventSemaphore(
                        name=f"bass_waitsplit_{seq}",
                        engine=inst.engine,
                        ins=[],
                        outs=[],
                        sync_info=mybir.SyncInfo(on_wait=[w], on_update=[]),
                    )
                    seq += 1
                    out_insts.append(carrier)
                inst.sync_info = mybir.SyncInfo(
                    on_wait=[waits[-1]], on_update=list(si.on_update or [])
                )
            out_insts.append(inst)
        blk.instructions = out_insts


def TileCtx(nc):
    return tile.TileContext(nc)


def _host_shards(x, W_qkv, b_qkv):
    """Build the 8 per-core input maps (numpy, host-side)."""
    x = np.asarray(x, dtype=np.float32)
    W = np.asarray(W_qkv, dtype=np.float32)
    bias = np.asarray(b_qkv, dtype=np.float32)
    xTs = [np.ascontiguousarray(x[b].T).astype(ml_dtypes.bfloat16) for b in range(B)]
    in_maps = []
    for c in range(NCORES):
        b, g = divmod(c, NCORES // B)  # batch, head-group
        h0 = HPC * g
        # wqkT columns: [Q_h0|Q_h1 .. interleaved by PAIR]: tile i (128 cols)
        # = [Q_{h0+2i}(64) | Q_{h0+2i+1}(64)] for i=0,1 then K pairs.
        qcols, kcols, bqcols = [], [], []
        for i in range(HPC // 2):
            for j in range(2):
                h = h0 + 2 * i + j
                qcols.append(W[DH * h : DH * (h + 1)])
                kcols.append(W[E + DH * h : E + DH * (h + 1)])
        for i in range(HPC // 2):
            bqcols.append(
                np.concatenate(
                    [
                        bias[DH * (h0 + 2 * i) : DH * (h0 + 2 * i + 1)],
                        bias[DH * (h0 + 2 * i + 1) : DH * (h0 + 2 * i + 2)],
                    ]
                )
            )
        wqk = np.concatenate(qcols + kcols, axis=0)  # [512, E]
        wqkT = np.ascontiguousarray(wqk.T).astype(ml_dtypes.bfloat16)
        wv = W[2 * E + DH * h0 : 2 * E + DH * (h0 + HPC)]  # [256, E]
        wvT = np.ascontiguousarray(wv.T).astype(ml_dtypes.bfloat16)
        bq8 = (np.stack(bqcols, axis=1) / np.sqrt(DH)).astype(ml_dtypes.bfloat16)
        bv = bias[2 * E + DH * h0 : 2 * E + DH * (h0 + HPC)].reshape(1, FV)
        in_maps.append(
            {
                "xT": xTs[b],
                "wqkT": wqkT,
                "wvT": wvT,
                "bq8": bq8,
                "bv": bv.astype(ml_dtypes.bfloat16),
            }
        )
    return in_maps


LAST_EXEC_NS = None


def kernel(x, W_qkv, b_qkv):
    global LAST_EXEC_NS
    in_maps = _host_shards(x, W_qkv, b_qkv)
    if "nc" not in _CACHED:
        _CACHED["nc"] = _build_nc()
    trace = bool(int(os.environ.get("BASS_KERNEL_TRACE", "0")))
    res = run_bass_kernel_spmd(
        _CACHED["nc"], in_maps, core_ids=list(range(NCORES)), trace=trace
    )
    LAST_EXEC_NS = res.exec_time_ns
    out = np.zeros((B, S, E), dtype=np.float32)
    for c in range(NCORES):
        b, g = divmod(c, NCORES // B)
        out[b, :, FV * g : FV * (g + 1)] = res.results[c]["out"]
    return out


# revision 42
# speedup vs baseline: 1.2584x; 1.0466x over previous
"""Multi-head attention (B=2, S=2048, E=1024, H=16, DH=64) on 8 TRN2 NeuronCores.

Sharding: core c handles batch c//4 and heads [4*(c%4), 4*(c%4)+4).
Each core computes its QKV projection slice and full attention for its 4
heads locally; host gathers the per-core [S, 256] outputs into [B, S, E].

Per-core kernel layout choices:
  - x is pre-transposed on host to xT [E, S]; W slices pre-transposed to
    [E, F].  QKV projection runs in two passes:
      pass 1 (transposed out): qkT [f, s] for Q,K — psum[f=128, s=512] =
        sum_e WqkT[e, f].T @ xT[e, s]; bias is added during the psum->sbuf
        copy as a per-partition tensor_scalar_add (bias varies along f).
      pass 2 (natural out): v [s, f] — psum[s=128, f=256] = bias seed +
        sum_e xT[e, s].T @ WvT[e, f]; bias seeded via a K=1 matmul
        (ones[1,128].T @ bv[1,256]).
  - Heads are paired in qkT tiles: tile rows 0:64 = head 2i, 64:128 = head
    2i+1.  Score matmuls slice partitions [0:64] or [64:128] of both
    operands (contraction d=64, tile_position rows 0/64).
  - Scores are computed transposed: sT[k, q] = K_tile.T @ QT, so softmax's
    denominator comes for free from the AV matmul via a ones column
    appended to V ([V | 1], N=65).  exp runs on ScalarE straight out of
    PSUM with scale=1/8 folded in; no max-subtraction (scores/8 are ~N(0,
    0.33), far from overflow).
  - AV: av[q, 0:65] += expT[k, q].T @ [V|1][k, 0:65] accumulated over 16
    k-tiles in PSUM; final normalize out[q, d] = av[q, d] * recip(av[q, 64])
    is a per-partition tensor_scalar_mul on VectorE.
"""

import os
from contextlib import ExitStack

import ml_dtypes
import numpy as np

import concourse.bass as bass
import concourse.mybir as mybir
import concourse.tile as tile
from concourse.bass import ts
from concourse.bass_utils import run_bass_kernel_spmd

B, S, E, H, DH = 2, 2048, 1024, 16, 64
NCORES = 8
HPC = H // (NCORES // B)  # heads per core = 4
FQK = HPC * 2 * DH  # 512: [Q_h0|K_h0|Q_h1|K_h1|...] pairs -> see layout below
FV = HPC * DH  # 256
ECH = E // 128  # 8 contraction chunks
ST = S // 128  # 16 s-tiles
BF16 = mybir.dt.bfloat16
F32 = mybir.dt.float32

_CACHED = {}


def _build_nc(
    loop: int | None = None,
    skip_qkv: bool = False,
    skip_exp: bool = False,
    skip_av: bool = False,
    interleave_av: bool = True,
) -> bass.Bass:
    nc = bass.Bass(trn_type="TRN2")
    xT = nc.dram_tensor("xT", [E, S], BF16, kind="ExternalInput")
    wqkT = nc.dram_tensor("wqkT", [E, FQK], BF16, kind="ExternalInput")
    wvT = nc.dram_tensor("wvT", [E, FV], BF16, kind="ExternalInput")
    # Q bias, pre-divided by sqrt(DH), head-pair layout (see _host_shards).
    # K bias is dropped entirely: adding b_k shifts every score in a softmax
    # row by a k-independent constant, which softmax cancels.  The Q bias
    # enters scores only through (b_q . K)[k], a per-k scalar we feed to the
    # exp activation as its per-partition bias AP.
    bq8 = nc.dram_tensor("bq8", [128, HPC // 2], BF16, kind="ExternalInput")
    bv = nc.dram_tensor("bv", [1, FV], BF16, kind="ExternalInput")
    out = nc.dram_tensor("out", [S, FV], F32, kind="ExternalOutput")

    QKT = FQK // 128  # 4 f-tiles for the Q/K pass; tile i holds heads 2i, 2i+1
    with TileCtx(nc) as tc, ExitStack() as ctx:
        if loop is not None:
            ctx.enter_context(tc.For_i(0, loop, 1))
        consts = ctx.enter_context(tc.tile_pool(name="consts", bufs=1))
        xt_pool = ctx.enter_context(tc.tile_pool(name="xt", bufs=1))
        qk_pool = ctx.enter_context(tc.tile_pool(name="qk", bufs=1))
        v_pool = ctx.enter_context(tc.tile_pool(name="v", bufs=1))
        out_pool = ctx.enter_context(tc.tile_pool(name="outp", bufs=1))
        expt_pool = ctx.enter_context(tc.tile_pool(name="expt", bufs=26))
        misc_pool = ctx.enter_context(tc.tile_pool(name="misc", bufs=2))

        # ---- constants / weights to SBUF ----
        ones = consts.tile([1, 128], BF16, tag="ones")
        nc.vector.memset(ones[:], 1.0)
        bq8_sb = consts.tile([128, HPC // 2], BF16, tag="bq8")
        nc.sync.dma_start(bq8_sb[:], bq8[:, :])
        bv_sb = consts.tile([1, FV], BF16, tag="bv")
        nc.sync.dma_start(bv_sb[:], bv[:, :])
        wqk_sb = consts.tile([128, ECH, FQK], BF16, tag="wqk")
        nc.sync.dma_start(wqk_sb[:], wqkT.rearrange("(c p) f -> p c f", p=128))
        wv_sb = consts.tile([128, ECH, FV], BF16, tag="wv")
        nc.sync.dma_start(wv_sb[:], wvT.rearrange("(c p) f -> p c f", p=128))

        xt_sb = xt_pool.tile([128, ECH, S], BF16, tag="xt")
        xt_r = xT.rearrange("(c p) s -> p c s", p=128)
        for c in range(ECH):
            nc.sync.dma_start(xt_sb[:, c, :], xt_r[:, c, :])

        # qkT storage: tile i [128, S]: rows 0:64 head 2i's Q^T (or K^T),
        # rows 64:128 head 2i+1's.  qk_sb[i] i in 0..3: [Q pair0, Q pair1,
        # K pair0, K pair1]  -- matches wqkT column layout (see host prep).
        qk_sb = qk_pool.tile([128, QKT, S], BF16, tag="qkt")
        if skip_qkv:
            nc.vector.memset(qk_sb[:, :, 0:2], 0.5)
        # v_sb: [128(k within tile), st, head, 65]; col 64 = 1.0
        v_sb = v_pool.tile([128, ST, HPC, DH + 1], BF16, tag="vsb")
        nc.vector.memset(v_sb[:, :, :, DH], 1.0)
        out_sb = out_pool.tile([128, ST, FV], F32, tag="outsb")
        if skip_av:
            nc.vector.memset(out_sb[:, 0, 0:2], 0.0)

        def emit_pass1_ft(pool, tag, ft):
            for sc in range(S // 512):
                ps = pool.tile([128, 512], F32, tag=tag, name="p1ps")
                for e in range(ECH):
                    nc.tensor.matmul(
                        ps[:],
                        lhsT=wqk_sb[:, e, ts(ft, 128)],
                        rhs=xt_sb[:, e, ts(sc, 512)],
                        start=(e == 0),
                        stop=(e == ECH - 1),
                    )
                nc.vector.tensor_copy(qk_sb[:, ft, ts(sc, 512)], ps[:])

        with tc.tile_pool(name="mmps", bufs=3, space="PSUM") as mmps:
            # pass 1, pair-0 tiles only (ft 0, 2): enough for head 0's
            # scores.  Pair 1 (ft 1, 3) is deferred into head 0's ACT-bound
            # exp window so ScalarE starts ~14us earlier.
            for ft in [0, 2] if not skip_qkv else []:
                emit_pass1_ft(mmps, "mm", ft)

        def emit_vpass(vmm, st):
            # pass 2 (V natural), one s-tile: seeded with the V bias.
            ps2 = vmm.tile([128, FV], F32, tag="vmm")
            nc.tensor.matmul(ps2[:], lhsT=ones[:], rhs=bv_sb[:], start=True, stop=False)
            for e in range(ECH):
                nc.tensor.matmul(
                    ps2[:],
                    lhsT=xt_sb[:, e, ts(st, 128)],
                    rhs=wv_sb[:, e, :],
                    start=False,
                    stop=(e == ECH - 1),
                )
            nc.vector.tensor_copy(
                v_sb[:, st, :, 0:DH], ps2.rearrange("p (h d) -> p h d", d=DH)
            )

        def head_slices(h):
            pair, lo = divmod(h, 2)
            qsl = slice(64 * lo, 64 * lo + 64)
            return pair, qsl

        def emit_bqk(scps, h):
            # (b_q . K)[k] / sqrt(DH): one [128,1] column per k-tile.
            pair, qsl = head_slices(h)
            k_t = qk_sb[qsl, 2 + pair, :]
            bqk_ps = scps.tile([128, ST], F32, tag="sc")
            for kt in range(ST):
                nc.tensor.matmul(
                    bqk_ps[:, kt : kt + 1],
                    lhsT=k_t[:, ts(kt, 128)],
                    rhs=bq8_sb[qsl, pair : pair + 1],
                    start=True,
                    stop=True,
                )
            bqk8 = misc_pool.tile([128, ST], F32, tag="bqk8", bufs=2)
            nc.vector.tensor_copy(bqk8[:], bqk_ps[:])
            return bqk8

        def emit_scores_exp(scps, h, kt, bqk8, expt):
            pair, qsl = head_slices(h)
            q_t = qk_sb[qsl, pair, :]
            k_t = qk_sb[qsl, 2 + pair, :]
            for qh in range(2):
                sc_ps = scps.tile([128, 1024], F32, tag="sc")
                for qq in range(2):
                    nc.tensor.matmul(
                        sc_ps[:, ts(qq, 512)],
                        lhsT=k_t[:, ts(kt, 128)],
                        rhs=q_t[:, ts(2 * qh + qq, 512)],
                        start=True,
                        stop=True,
                    )
                if not skip_exp:
                    nc.scalar.activation(
                        expt[:, ts(qh, 1024)],
                        sc_ps[:],
                        mybir.ActivationFunctionType.Exp,
                        bias=bqk8[:, kt : kt + 1],
                        scale=float(1.0 / np.sqrt(DH)),
                    )

        def emit_av(av, h, expts, qts):
            # qt-outer / kt-inner: PSUM has_written clears at bank
            # granularity on start=True, so accumulation groups sharing a
            # bank must run one-at-a-time, not interleaved.
            for qt in qts:
                for kt in range(ST):
                    nc.tensor.matmul(
                        av[:, qt, 0 : DH + 1],
                        lhsT=expts[kt][:, ts(qt, 128)],
                        rhs=v_sb[:, kt, h, :],
                        start=(kt == 0),
                        stop=(kt == ST - 1),
                    )

        def emit_normalize(av, h):
            rec = misc_pool.tile([128, ST], F32, tag="rec", bufs=2)
            nc.vector.reciprocal(rec[:], av[:, :, DH])
            # Copy then in-place multiply: keeps each DVE instruction's
            # sync waits on a single proc (PE for the copy, DVE for the
            # mul) — DVE ops only have one HW wait slot.
            for qt in range(ST):
                nc.vector.tensor_copy(out_sb[:, qt, ts(h, DH)], av[:, qt, 0:DH])
                nc.vector.tensor_mul(
                    out_sb[:, qt, ts(h, DH)],
                    out_sb[:, qt, ts(h, DH)],
                    rec[:, qt : qt + 1].broadcast_to((128, DH)),
                )

        # Attention, software-pipelined one head deep: head h's ACT-bound
        # exp loop hides head h-1's AV matmuls (2 qt-groups per kt over the
        # first 8 kt) and, for h=0, the V projection pass (one s-tile per
        # kt).  PSUM: sc 2x2 banks + (vmm 2 banks during h0 | av 4 banks
        # from h1 on) <= 8.
        with tc.tile_pool(name="scps", bufs=2, space="PSUM") as scps:
            inner = ExitStack()
            vmm = inner.enter_context(tc.tile_pool(name="vmm", bufs=2, space="PSUM"))
            avps = None
            expts_by_head = {}
            av_by_head = {}
            for h in range(HPC):
                bqk8 = emit_bqk(scps, h)
                expts_by_head[h] = []
                prev = h - 1 if h > 0 and interleave_av and not skip_av else None
                if prev is not None:
                    av_by_head[prev] = avps.tile([128, ST, 128], F32, tag="av", name="av")
                for kt in range(ST):
                    expt = expt_pool.tile([128, S], BF16, tag="expt")
                    expts_by_head[h].append(expt)
                    if skip_exp:
                        nc.vector.memset(expt[:, 0:2], 0.5)
                    emit_scores_exp(scps, h, kt, bqk8, expt)
                    if h == 0 and not skip_qkv:
                        emit_vpass(vmm, kt)
                    if prev is not None and kt < 8:
                        emit_av(
                            av_by_head[prev], prev, expts_by_head[prev], [2 * kt, 2 * kt + 1]
                        )
                    if prev is not None and kt == 8:
                        emit_normalize(av_by_head[prev], prev)
                        del expts_by_head[prev]
                if h == 0:
                    # Deferred pass-1 pair 1: PE chews through these while
                    # ACT is still on head 0's exps; ready before head 1.
                    if not skip_qkv:
                        emit_pass1_ft(vmm, "vmm", 1)
                        emit_pass1_ft(vmm, "vmm", 3)
                    # vmm's 2 banks are released before avps' 4 are claimed.
                    inner.close()
                    inner = ExitStack()
                    avps = inner.enter_context(
                        tc.tile_pool(name="avps", bufs=1, space="PSUM")
                    )
                if not interleave_av and not skip_av:
                    av_by_head[h] = avps.tile([128, ST, 128], F32, tag="av", name="av")
                    emit_av(av_by_head[h], h, expts_by_head[h], list(range(ST)))
                    emit_normalize(av_by_head[h], h)
                    del expts_by_head[h]
            if interleave_av and not skip_av:
                last = HPC - 1
                av_by_head[last] = avps.tile([128, ST, 128], F32, tag="av", name="av")
                emit_av(av_by_head[last], last, expts_by_head[last], list(range(ST)))
                emit_normalize(av_by_head[last], last)
            inner.close()

        nc.sync.dma_start(out.rearrange("(t p) f -> p t f", p=128), out_sb[:])
    _split_multiwaits(nc)
    return nc


_NO_SPLIT = ("InstEventSemaphore", "InstCall", "InstUnconditionalBranch")


def _split_multiwaits(nc: bass.Bass):
    """Walrus codegen rejects TPB instructions carrying >1 sync wait ("Too
    many sync wait commands"), but Tile's wait assigner can emit 2-3 when an
    instruction depends on several procs.  Hoist all but the last wait onto
    engine-level InstEventSemaphore carriers (the same construct Tile's own
    barriers use) inserted just before the instruction."""
    seq = 0
    for blk in nc.m.functions[0].blocks:
        out_insts = []
        for inst in blk.instructions:
            si = inst.sync_info
            waits = list(si.on_wait) if si is not None and si.on_wait else []
            if len(waits) > 1 and type(inst).__name__ not in _NO_SPLIT:
                for w in waits[:-1]:
                    carrier = mybir.InstEventSemaphore(
                        name=f"bass_waitsplit_{seq}",
                        engine=inst.engine,
                        ins=[],
                        outs=[],
                        sync_info=mybir.SyncInfo(on_wait=[w], on_update=[]),
                    )
                    seq += 1
                    out_insts.append(carrier)
                inst.sync_info = mybir.SyncInfo(
                    on_wait=[waits[-1]], on_update=list(si.on_update or [])
                )
            out_insts.append(inst)
        blk.instructions = out_insts


def TileCtx(nc):
    return tile.TileContext(nc)


def _host_shards(x, W_qkv, b_qkv):
    """Build the 8 per-core input maps (numpy, host-side)."""
    x = np.asarray(x, dtype=np.float32)
    W = np.asarray(W_qkv, dtype=np.float32)
    bias = np.asarray(b_qkv, dtype=np.float32)
    xTs = [np.ascontiguousarray(x[b].T).astype(ml_dtypes.bfloat16) for b in range(B)]
    in_maps = []
    for c in range(NCORES):
        b, g = divmod(c, NCORES // B)  # batch, head-group
        h0 = HPC * g
        # wqkT columns: [Q_h0|Q_h1 .. interleaved by PAIR]: tile i (128 cols)
        # = [Q_{h0+2i}(64) | Q_{h0+2i+1}(64)] for i=0,1 then K pairs.
        qcols, kcols, bqcols = [], [], []
        for i in range(HPC // 2):
            for j in range(2):
                h = h0 + 2 * i + j
                qcols.append(W[DH * h : DH * (h + 1)])
                kcols.append(W[E + DH * h : E + DH * (h + 1)])
        for i in range(HPC // 2):
            bqcols.append(
                np.concatenate(
                    [
                        bias[DH * (h0 + 2 * i) : DH * (h0 + 2 * i + 1)],
                        bias[DH * (h0 + 2 * i + 1) : DH * (h0 + 2 * i + 2)],
                    ]
                )
            )
        wqk = np.concatenate(qcols + kcols, axis=0)  # [512, E]
        wqkT = np.ascontiguousarray(wqk.T).astype(ml_dtypes.bfloat16)
        wv = W[2 * E + DH * h0 : 2 * E + DH * (h0 + HPC)]  # [256, E]
        wvT = np.ascontiguousarray(wv.T).astype(ml_dtypes.bfloat16)
        bq8 = (np.stack(bqcols, axis=1) / np.sqrt(DH)).astype(ml_dtypes.bfloat16)
        bv = bias[2 * E + DH * h0 : 2 * E + DH * (h0 + HPC)].reshape(1, FV)
        in_maps.append(
            {
                "xT": xTs[b],
                "wqkT": wqkT,
                "wvT": wvT,
                "bq8": bq8,
                "bv": bv.astype(ml_dtypes.bfloat16),
            }
        )
    return in_maps


LAST_EXEC_NS = None


def kernel(x, W_qkv, b_qkv):
    global LAST_EXEC_NS
    in_maps = _host_shards(x, W_qkv, b_qkv)
    if "nc" not in _CACHED:
        _CACHED["nc"] = _build_nc()
    trace = bool(int(os.environ.get("BASS_KERNEL_TRACE", "0")))
    res = run_bass_kernel_spmd(
        _CACHED["nc"], in_maps, core_ids=list(range(NCORES)), trace=trace
    )
    LAST_EXEC_NS = res.exec_time_ns
    out = np.zeros((B, S, E), dtype=np.float32)
    for c in range(NCORES):
        b, g = divmod(c, NCORES // B)
        out[b, :, FV * g : FV * (g + 1)] = res.results[c]["out"]
    return out
